# revision 24
# baseline (speedup 1.0000x reference)
"""Covariance pooling kernel for Trainium2 (8 NeuronCores, data-parallel over batch).

y[b] = (1/M) * (x[b] - mean(x[b])) @ (x[b] - mean(x[b]))^T  with x[b] [C=128, M=4096].

Strategy per core (8 batches/core):
  - stream each batch tile from HBM, casting fp32 -> bf16 during the DMA
    (SWDGE cast): PE then streams 1 col/cycle everywhere (fp32 pays 2)
  - PE-transpose each 128-wide bf16 chunk (identity matmul) -> PSUM
  - copy transposed chunk PSUM->SBUF (alternating DVE/ACT), with a constant
    ones column appended so the Gram matmul also accumulates the row-sum s
  - accumulate G = sum_k Xk^T.T @ [Xk^T | 1] into fp32 PSUM ([C, C+1] = [G | s])
  - epilogue (fp32): y = (G - s s^T / M) / M via a rank-1 PSUM-accumulated
    correction; s_col -> s_row via a small PE transpose with an fp32 identity

Sync-wait discipline: most compute instructions hold a single wait command
(bacc splits the rest into event semaphores); the kernel is arranged so hot
instructions need at most one foreign-semaphore wait.
"""

import numpy as np

import ml_dtypes
import concourse.bass as bass
import concourse.tile as tile
from concourse import bacc, mybir
from concourse.bass_utils import run_bass_kernel_spmd

N_CORES = 8
B_FULL = 64
B_CORE = B_FULL // N_CORES  # 8 batches per core
C = 128
M = 4096  # 64*64 spatial
CHUNKS = M // 128  # 32
F32 = mybir.dt.float32
BF16 = mybir.dt.bfloat16
COPY = mybir.ActivationFunctionType.Copy

_CACHE: dict = {}


def _build_program() -> bass.Bass:
    nc = bacc.Bacc()
    x = nc.declare_dram_parameter("x", [B_CORE, C, M], F32, isOutput=False)
    ident = nc.declare_dram_parameter("ident", [C, C], mybir.dt.bfloat16, isOutput=False)
    y = nc.declare_dram_parameter("y", [B_CORE, C, C], F32, isOutput=True)

    NSLOT = 24  # transposed-chunk slots in flight (6 groups of 4)

    with tile.TileContext(nc) as tc:
        with (
            tc.tile_pool(name="singles", bufs=1) as singles,
            tc.tile_pool(name="xin", bufs=6) as xin_pool,
            tc.tile_pool(name="yout", bufs=3) as yout_pool,
            tc.tile_pool(name="small", bufs=4) as small_pool,
            tc.tile_pool(name="tp", bufs=4, space="PSUM") as tp_pool,
            tc.tile_pool(name="gram", bufs=3, space="PSUM") as gram_pool,
            tc.tile_pool(name="srow", bufs=1, space="PSUM") as srow_pool,
        ):
            # identity arrives by HWDGE DMA (keeps the GpSimd queue free so
            # the SWDGE input stream starts as early as possible)
            identity = singles.tile([128, 128], BF16)
            nc.sync.dma_start(identity, ident[:, :])

            # Transposed-chunk ring buffer; col 128 holds the constant 1.0
            # column that makes the Gram matmul also produce the row-sums.
            xt = singles.tile([128, NSLOT, 132], BF16)
            nc.vector.memset(xt[:, :, 128:129], 1.0)

            # PE warm-up: absorbs the wait on identity (GpSimd) so the first
            # real transpose only waits on its input DMA (one wait slot).
            warm = tp_pool.tile([128, 4, 128], BF16, tag="tp")
            nc.tensor.transpose(warm[:, 0], identity, identity)

            for b in range(B_CORE):
                x_tile = xin_pool.tile([128, M], BF16)
                # SWDGE cast DMA: fp32 in HBM -> bf16 in SBUF, split so
                # transposes start early and WAR releases are finer-grained.
                nsplit = 4
                step = M // nsplit
                for h in range(nsplit):
                    nc.gpsimd.dma_start(
                        x_tile[:, h * step : (h + 1) * step],
                        x[b][:, h * step : (h + 1) * step],
                    )

                gram = gram_pool.tile([128, 129], F32)
                # 4-chunk groups: 4 PE transposes into one PSUM tile, then one
                # strided PSUM->SBUF copy moves all 4 transposed chunks.
                for g in range(CHUNKS // 4):
                    tp = tp_pool.tile([128, 4, 128], BF16, tag="tp")
                    for j in range(4):
                        k = g * 4 + j
                        nc.tensor.transpose(
                            tp[:, j], x_tile[:, k * 128 : (k + 1) * 128], identity
                        )
                    s0 = (g % 6) * 4
                    dst = xt[:, s0 : s0 + 4, 0:128]
                    nc.vector.tensor_copy(dst, tp)
                    for j in range(4):
                        k = g * 4 + j
                        slot = s0 + j
                        nc.tensor.matmul(
                            gram,
                            xt[:, slot, 0:128],
                            xt[:, slot, 0:129],
                            start=(k == 0),
                            stop=False,
                        )

                # epilogue: y = (G - s s^T / M) / M
                # All gram-PSUM readers stay on DVE so the WAR release of the
                # PSUM slot merges with the next batch's DVE waits.
                s_col = small_pool.tile([128, 1], BF16)
                nc.vector.tensor_copy(s_col, gram[:, 128:129])
                s_row_ps = srow_pool.tile([1, 128], BF16)
                nc.tensor.transpose(s_row_ps, s_col, identity)
                srow = small_pool.tile([1, 128], BF16)
                srow_neg = small_pool.tile([1, 128], BF16)
                nc.vector.tensor_copy(srow, s_row_ps)
                nc.vector.tensor_scalar_mul(srow_neg, s_row_ps, -1.0 / M)
                nc.tensor.matmul(gram[:, 0:128], srow, srow_neg, start=False, stop=True)

                y_tile = yout_pool.tile([128, 128], F32)
                nc.vector.tensor_scalar_mul(y_tile, gram[:, 0:128], 1.0 / M)
                nc.sync.dma_start(y[b], y_tile)

    nc.compile()  # bacc passes: split multi-waits into event semaphores etc.
    return nc


def _get_program() -> bass.Bass:
    if "nc" not in _CACHE:
        _CACHE["nc"] = _build_program()
    return _CACHE["nc"]


def _run(x: np.ndarray, **spmd_kwargs):
    x = np.ascontiguousarray(np.asarray(x), dtype=np.float32)
    assert x.shape == (B_FULL, C, 64, 64), x.shape
    xf = x.reshape(B_FULL, C, M)
    shards = np.split(xf, N_CORES, axis=0)
    ident = np.eye(C, dtype=ml_dtypes.bfloat16)
    in_maps = [{"x": s, "ident": ident} for s in shards]
    nc = _get_program()
    res = run_bass_kernel_spmd(nc, in_maps, list(range(N_CORES)), **spmd_kwargs)
    out = np.concatenate([res.results[i]["y"] for i in range(N_CORES)], axis=0)
    return out, res


def kernel(x: np.ndarray) -> np.ndarray:
    out, _ = _run(x)
    return out


# revision 25
# speedup vs baseline: 1.0120x; 1.0120x over previous
"""Covariance pooling kernel for Trainium2 (8 NeuronCores, data-parallel over batch).

y[b] = (1/M) * (x[b] - mean(x[b])) @ (x[b] - mean(x[b]))^T  with x[b] [C=128, M=4096].

Strategy per core (8 batches/core):
  - stream each batch tile from HBM, casting fp32 -> bf16 during the DMA
    (SWDGE cast): PE then streams 1 col/cycle everywhere (fp32 pays 2)
  - PE-transpose each 128-wide bf16 chunk (identity matmul) -> PSUM
  - copy transposed chunk PSUM->SBUF (alternating DVE/ACT), with a constant
    ones column appended so the Gram matmul also accumulates the row-sum s
  - accumulate G = sum_k Xk^T.T @ [Xk^T | 1] into fp32 PSUM ([C, C+1] = [G | s])
  - epilogue (fp32): y = (G - s s^T / M) / M via a rank-1 PSUM-accumulated
    correction; s_col -> s_row via a small PE transpose with an fp32 identity

Sync-wait discipline: most compute instructions hold a single wait command
(bacc splits the rest into event semaphores); the kernel is arranged so hot
instructions need at most one foreign-semaphore wait.
"""

import numpy as np

import ml_dtypes
import concourse.bass as bass
import concourse.tile as tile
from concourse import bacc, mybir
from concourse.bass_utils import run_bass_kernel_spmd

N_CORES = 8
B_FULL = 64
B_CORE = B_FULL // N_CORES  # 8 batches per core
C = 128
M = 4096  # 64*64 spatial
CHUNKS = M // 128  # 32
F32 = mybir.dt.float32
BF16 = mybir.dt.bfloat16
COPY = mybir.ActivationFunctionType.Copy

_CACHE: dict = {}


def _build_program() -> bass.Bass:
    nc = bacc.Bacc()
    x = nc.declare_dram_parameter("x", [B_CORE, C, M], F32, isOutput=False)
    ident = nc.declare_dram_parameter("ident", [C, C], mybir.dt.bfloat16, isOutput=False)
    y = nc.declare_dram_parameter("y", [B_CORE, C, C], F32, isOutput=True)

    NSLOT = 24  # transposed-chunk slots in flight (6 groups of 4)

    with tile.TileContext(nc) as tc:
        with (
            tc.tile_pool(name="singles", bufs=1) as singles,
            tc.tile_pool(name="xin", bufs=6) as xin_pool,
            tc.tile_pool(name="yout", bufs=3) as yout_pool,
            tc.tile_pool(name="small", bufs=4) as small_pool,
            tc.tile_pool(name="tp", bufs=4, space="PSUM") as tp_pool,
            tc.tile_pool(name="gram", bufs=3, space="PSUM") as gram_pool,
            tc.tile_pool(name="srow", bufs=1, space="PSUM") as srow_pool,
        ):
            # identity arrives by HWDGE DMA (keeps the GpSimd queue free so
            # the SWDGE input stream starts as early as possible)
            identity = singles.tile([128, 128], BF16)
            nc.sync.dma_start(identity, ident[:, :])

            # Transposed-chunk ring buffer; col 128 holds the constant 1.0
            # column that makes the Gram matmul also produce the row-sums.
            xt = singles.tile([128, NSLOT, 132], BF16)
            nc.vector.memset(xt[:, :, 128:129], 1.0)

            # PE warm-up: absorbs the wait on identity (GpSimd) so the first
            # real transpose only waits on its input DMA (one wait slot).
            warm = tp_pool.tile([128, 4, 128], BF16, tag="tp")
            nc.tensor.transpose(warm[:, 0], identity, identity)

            for b in range(B_CORE):
                x_tile = xin_pool.tile([128, M], BF16)
                # SWDGE cast DMA: fp32 in HBM -> bf16 in SBUF, split so
                # transposes start early and WAR releases are finer-grained.
                nsplit = 4
                step = M // nsplit
                for h in range(nsplit):
                    nc.gpsimd.dma_start(
                        x_tile[:, h * step : (h + 1) * step],
                        x[b][:, h * step : (h + 1) * step],
                    )

                gram = gram_pool.tile([128, 129], F32)
                # 4-chunk groups: 4 PE transposes into one PSUM tile, then one
                # strided PSUM->SBUF copy moves all 4 transposed chunks.
                for g in range(CHUNKS // 4):
                    tp = tp_pool.tile([128, 4, 128], BF16, tag="tp")
                    for j in range(4):
                        k = g * 4 + j
                        nc.tensor.transpose(
                            tp[:, j], x_tile[:, k * 128 : (k + 1) * 128], identity
                        )
                    s0 = (g % 6) * 4
                    dst = xt[:, s0 : s0 + 4, 0:128]
                    if g % 2 == 0:
                        nc.vector.tensor_copy(dst, tp)
                    else:
                        nc.scalar.activation(dst, tp, COPY)
                    for j in range(4):
                        k = g * 4 + j
                        slot = s0 + j
                        nc.tensor.matmul(
                            gram,
                            xt[:, slot, 0:128],
                            xt[:, slot, 0:129],
                            start=(k == 0),
                            stop=False,
                        )

                # epilogue: y = (G - s s^T / M) / M
                # All gram-PSUM readers stay on DVE so the WAR release of the
                # PSUM slot merges with the next batch's DVE waits.
                s_col = small_pool.tile([128, 1], BF16)
                nc.vector.tensor_copy(s_col, gram[:, 128:129])
                s_row_ps = srow_pool.tile([1, 128], BF16)
                nc.tensor.transpose(s_row_ps, s_col, identity)
                srow = small_pool.tile([1, 128], BF16)
                srow_neg = small_pool.tile([1, 128], BF16)
                nc.vector.tensor_copy(srow, s_row_ps)
                nc.vector.tensor_scalar_mul(srow_neg, s_row_ps, -1.0 / M)
                nc.tensor.matmul(gram[:, 0:128], srow, srow_neg, start=False, stop=True)

                y_tile = yout_pool.tile([128, 128], F32)
                nc.vector.tensor_scalar_mul(y_tile, gram[:, 0:128], 1.0 / M)
                nc.sync.dma_start(y[b], y_tile)

    nc.compile()  # bacc passes: split multi-waits into event semaphores etc.
    return nc


def _get_program() -> bass.Bass:
    if "nc" not in _CACHE:
        _CACHE["nc"] = _build_program()
    return _CACHE["nc"]


def _run(x: np.ndarray, **spmd_kwargs):
    x = np.ascontiguousarray(np.asarray(x), dtype=np.float32)
    assert x.shape == (B_FULL, C, 64, 64), x.shape
    xf = x.reshape(B_FULL, C, M)
    shards = np.split(xf, N_CORES, axis=0)
    ident = np.eye(C, dtype=ml_dtypes.bfloat16)
    in_maps = [{"x": s, "ident": ident} for s in shards]
    nc = _get_program()
    res = run_bass_kernel_spmd(nc, in_maps, list(range(N_CORES)), **spmd_kwargs)
    out = np.concatenate([res.results[i]["y"] for i in range(N_CORES)], axis=0)
    return out, res


def kernel(x: np.ndarray) -> np.ndarray:
    out, _ = _run(x)
    return out
